# revision 48
# baseline (speedup 1.0000x reference)
"""Single-head causal attention (B=8, S=2048, E=2048, D=128) on 8 trn2 cores.

Sharding: data-parallel over batch — one batch element per NeuronCore.

Host marshaling per core: xT = x[b].T in two precisions — fp8(e4m3) for
the q/k projections and fp16 for the v projection; q/k weights are
pre-scaled by W_SCALE, transposed and packed into one fp8 tensor in the
exact SBUF tile layout, the v weights into one fp16 tensor, the three
biases into one [128, 3] tensor.

Per-core dataflow (f32 PSUM accumulation everywhere):
  - q/k projections run fp8 DoubleRow (2 contraction chunks per matmul,
    ~1.4x PE rate); the 1/W_SCALE unscale + bias fold into the VectorE
    PSUM->SBUF evacuation. v projection runs fp16 (10-bit mantissa) —
    v quantization error hits the output directly, while q/k error is
    crushed by the 1/sqrt(2048) softmax logit scale (rel err ~3.5e-3).
  - vT is re-transposed on the PE into natural [S, D] blocks (4 per
    s-tile into one PSUM bank, one strided DVE evac), augmented with a
    ones column: the AV matmul then yields the softmax denominator for
    free as output column 128
  - scoresT[k, q] per k-block j: single fp16 matmul (K=D=128), exact
    causal trim of the q range; the diagonal 128-block's causal mask is
    accumulated on the PE (extra N=128 matmul, fp16-safe -60000);
    ScalarE computes exp(scale*s) straight out of PSUM into fp16 probsT
  - AV per q-block i accumulates probsT_j.T @ v_aug_j over j<=i in PSUM;
    VectorE takes 1/denominator and applies it during the final
    evacuation; DMA stores the natural-layout [128, 128] f32 result
  - emission order pipelines s-tiles: [qk-proj][AV of prev chunk]
    [scores][v-proj][vtrans]; AV(c) runs one s-tile late so exp(c)
    overlaps PE work instead of stalling before its own AV. For
    repeated execution, LOOP_UNROLL bodies share one For_i iteration so
    the loop's all-engine barrier fires half as often and body n+1's
    DMA/projections overlap body n's exp/AV tail.
"""

import math

import numpy as np

B = 8
S = 2048
E = 2048
D = 128
P = 128
NE = E // P  # 16 contraction chunks
NS = S // P  # 16 sequence blocks
ST = 512  # s-tile width for projections / score chunks
NST = S // ST  # 4
VW = D + 1  # logical v block width incl. ones column
VSTRIDE = D + 1  # physical stride of v blocks in SBUF
SCALE = 1.0 / math.sqrt(S)
NEG = -1.0e30

_PROGRAMS = {}

# which phases to emit (for microbenchmarking): subset of
# {"proj", "vtrans", "scores", "av", "store"}
PHASES = frozenset({"proj", "vtrans", "scores", "av", "store"})

# matmul operand precision: "fp16" (10-bit mantissa, same PE speed) /
# "bf16" / "fp8" (DoubleRow projections, 2x PE, rel err 2.95e-2: FAILS) /
# "mixed" (q/k fp8 DoubleRow + v fp16: q/k quantization error is crushed
# by the 1/sqrt(2048) softmax logit scale, v keeps fp16 accuracy ->
# rel err ~3.6e-3, ~2/3 of the all-fp8 PE win)
PROJ_DTYPE = "mixed"
W_SCALE = 256.0  # host pre-scale of W before fp8 quantization

# tunables: PSUM bank split (proj, sc, out; vt fixed at 1) and probs pool depth
PS_CFG = (2, 3, 2)
PPOOL = 32  # avmid keeps two chunks' probs live (+4 cross-body slack)
OUT_DMA = "sync"  # "sync" | "scalar": DGE queue for output stores
# (scalar steals ~667ns of Act sequencer per store from the exp stream;
#  DVE/PE cannot initiate DMAs)
SPLIT_SCORES = False  # legacy order only
XBUFS8 = 5  # fp8 x-tile pool depth (1 slot of cross-body overlap)
XBUFS16 = 5  # fp16 x-tile pool depth
FINE = 2  # first-tile DMA piece count (each dma_start costs ~625ns ring gen)
# "avmid": per st emit [qk][AV(st-1)][scores][v][vtrans], tail AV(3) --
# AV(st-1) covers the q/k PSUM-evac bubble and exp(c) overlaps v/vtrans/
# next qk instead of serializing before its own AV. "legacy": baseline.
ORDER = "avmid"
X16_RING = "alt"  # "sync" | "scalar" | "alt": DGE ring(s) for the xt16 stream
# fold the causal mask into the PE score accumulation (one extra N=128
# matmul on the diagonal piece) instead of a DVE add -- keeps the DVE off
# the scores->exp critical path. Mask value -60000 is fp16-safe and
# underflows exp to exactly 0 after the 1/sqrt(S) scale.
CMASK_PE = True
# "host": DMA a separate fp8 copy of x (12.6MB/iter total input DMA).
# "cast": DMA only the fp16 copy and produce the fp8 q/k operand with
# DVE tensor_copy casts on device (8.6MB/iter; DVE has the slack).
X8_SRC = "host"
FINE16 = 2  # DMA/cast piece count for xt16 tiles st>=1 (cast mode)
# bodies per For_i iteration: the loop's all-engine barrier fires every
# LOOP_UNROLL bodies, so body n+1's DMA stream and projections overlap
# body n's exp/AV tail. iters is decomposed as
# For_i(iters//LOOP_UNROLL) * LOOP_UNROLL + (iters % LOOP_UNROLL) bodies.
LOOP_UNROLL = 8
# "pe": transpose vT->v on the PE (transpose-mode matmul + DVE evac).
# "dma": XBAR hardware transpose on the DMA engines -- frees PE time and
# the vt PSUM bank (enables sc_ps=4 via PS_CFG=(2,4,2)).
VT_MODE = "pe"
# "block": one dma_start per 128-row output block (16/body).
# "chunk": one strided dma_start per 512-row chunk (4/body).
# "body": ONE strided dma_start for the whole body's output -- each
# dma_start costs ~625ns of DGE ring generation that sits between this
# body's input stream and the next body's.
OSTORE = "body"


def _modes():
    """per-projection matmul mode: 'dr' (fp8 DoubleRow) or '16'."""
    if PROJ_DTYPE == "fp8":
        return {"q": "dr", "k": "dr", "v": "dr"}
    if PROJ_DTYPE == "mixed":
        return {"q": "dr", "k": "dr", "v": "16"}
    return {"q": "16", "k": "16", "v": "16"}


def build_program(iters=1, unroll=1):
    global _PROGRAMS
    key = (iters, unroll, PHASES, PROJ_DTYPE, PS_CFG, PPOOL, OUT_DMA,
           SPLIT_SCORES, VSTRIDE, XBUFS8, XBUFS16, FINE, ORDER, X16_RING,
           CMASK_PE, X8_SRC, FINE16, LOOP_UNROLL, VT_MODE, OSTORE)
    if key in _PROGRAMS:
        return _PROGRAMS[key]

    import contextlib

    import concourse.bacc as bacc
    import concourse.mybir as mybir
    import concourse.tile as tile
    from concourse.masks import make_identity

    f32 = mybir.dt.float32
    f8 = mybir.dt.float8e4
    modes = _modes()
    mmdt = mybir.dt.bfloat16 if PROJ_DTYPE == "bf16" else mybir.dt.float16
    need8 = any(m == "dr" for m in modes.values())
    need16 = any(m == "16" for m in modes.values())
    n8 = sum(1 for m in modes.values() if m == "dr")
    n16 = 3 - n8

    nc = bacc.Bacc("TRN2", target_bir_lowering=False, debug=False)

    # x arrives host-packed as [P, NST*NE*ST] so each s-tile load is one
    # fully-contiguous per-partition slice (max DMA line rate)
    dram = {"out": nc.dram_tensor("out", [S, D], f32, kind="ExternalOutput")}
    if need8:
        if X8_SRC == "host":
            dram["xT8"] = nc.dram_tensor("xT8", [P, NST * NE * ST], f8, kind="ExternalInput")
        dram["w8"] = nc.dram_tensor("w8", [P, n8 * NE * D], f8, kind="ExternalInput")
    if need16 or (need8 and X8_SRC == "cast"):
        dram["xT16"] = nc.dram_tensor("xT16", [P, NST * NE * ST], mmdt, kind="ExternalInput")
    if need16:
        dram["w16"] = nc.dram_tensor("w16", [P, n16 * NE * D], mmdt, kind="ExternalInput")
    bias_d = nc.dram_tensor("bias", [D, 3], f32, kind="ExternalInput")

    with tile.TileContext(nc) as tc:
        with (
            tc.tile_pool(name="const", bufs=1) as cpool,
            tc.tile_pool(name="xt8", bufs=XBUFS8) as xpool8,
            tc.tile_pool(name="xt16", bufs=XBUFS16) as xpool16,
            tc.tile_pool(name="qkv", bufs=1) as qkvpool,
            tc.tile_pool(name="probs", bufs=PPOOL) as ppool,
            tc.tile_pool(name="osb", bufs=2) as opool,
            tc.tile_pool(name="misc", bufs=2) as mpool,
            tc.tile_pool(name="proj_ps", bufs=PS_CFG[0], space="PSUM") as proj_ps,
            tc.tile_pool(name="sc_ps", bufs=PS_CFG[1], space="PSUM") as sc_ps,
            tc.tile_pool(name="vt_ps", bufs=1, space="PSUM") as vt_ps,
            tc.tile_pool(name="out_ps", bufs=PS_CFG[2], space="PSUM") as out_ps,
        ):
            # ---- iteration-invariant setup ----
            ident = cpool.tile([P, P], mmdt, tag="ident")
            make_identity(nc, ident[:])
            # cmaskT[k_local, q_local]: 0 where q >= k (valid), NEG where q < k
            cmaskT = cpool.tile([P, P], mmdt if CMASK_PE else f32, tag="cmaskT")
            nc.gpsimd.memset(cmaskT[:], 0.0)
            nc.gpsimd.affine_select(
                out=cmaskT[:],
                in_=cmaskT[:],
                compare_op=mybir.AluOpType.is_ge,
                fill=-60000.0 if CMASK_PE else NEG,
                base=0,
                # iota[r, c] = c - r ; keep (0.0) where c - r >= 0
                pattern=[[1, P]],
                channel_multiplier=-1,
            )

            w_sb = {}
            b_sb = {}
            i8 = i16 = 0
            if need8:
                w8_sb = cpool.tile([P, n8 * NE * D], f8, tag="w8")
                nc.sync.dma_start(w8_sb[:], dram["w8"][:, :])
            if need16:
                w16_sb = cpool.tile([P, n16 * NE * D], mmdt, tag="w16")
                nc.sync.dma_start(w16_sb[:], dram["w16"][:, :])
            for pj in ("q", "k", "v"):
                if modes[pj] == "dr":
                    w_sb[pj] = w8_sb[:, i8 * NE * D : (i8 + 1) * NE * D]
                    i8 += 1
                else:
                    w_sb[pj] = w16_sb[:, i16 * NE * D : (i16 + 1) * NE * D]
                    i16 += 1
            bias_sb = cpool.tile([P, 3], f32, tag="bias")
            nc.sync.dma_start(bias_sb[:], bias_d[:, :])
            for n, pj in enumerate(("q", "k", "v")):
                b_sb[pj] = bias_sb[:, n : n + 1]

            # qkv destination tiles persist across iterations (bufs=1);
            # the ones column of v_aug is iteration-invariant -> init once
            qT_sb = qkvpool.tile([P, S], mmdt, tag="qT")
            kT_sb = qkvpool.tile([P, S], mmdt, tag="kT")
            vT_sb = qkvpool.tile([P, S], mmdt, tag="vT")
            v_sb = qkvpool.tile([P, NS * VSTRIDE], mmdt, tag="v")
            for sb in range(NS):
                nc.vector.memset(
                    v_sb[:, sb * VSTRIDE + D : sb * VSTRIDE + D + 1], 1.0
                )

            # HAM warmup while the first DMAs land (runs once, cold)
            wps = proj_ps.tile([P, ST], f32, name="warm", tag="proj")
            for wi in range(40):
                nc.tensor.matmul(
                    wps[:, 0:P],
                    lhsT=ident[:],
                    rhs=ident[:],
                    start=(wi == 0),
                    stop=(wi == 39),
                )

            def body():
                _emit_body(
                    nc,
                    mybir,
                    pools={
                        "xpool8": xpool8,
                        "xpool16": xpool16,
                        "qkvpool": qkvpool,
                        "ppool": ppool,
                        "opool": opool,
                        "mpool": mpool,
                        "proj_ps": proj_ps,
                        "sc_ps": sc_ps,
                        "vt_ps": vt_ps,
                        "out_ps": out_ps,
                    },
                    dram=dram,
                    consts={
                        "ident": ident,
                        "cmaskT": cmaskT,
                        "w": w_sb,
                        "b": b_sb,
                        "qkv": (qT_sb, kT_sb, vT_sb, v_sb),
                    },
                )

            hints = (
                mybir.EngineType.PE,
                mybir.EngineType.Activation,
                mybir.EngineType.DVE,
                mybir.EngineType.SP,
                mybir.EngineType.Pool,
            )
            if iters > 1:
                # LOOP_UNROLL bodies per hardware-loop iteration: For_i's
                # all-engine barrier fires once per group, so body n+1's
                # DMA/projections overlap body n's exp/AV tail
                pairs, rem = divmod(iters, LOOP_UNROLL)
                if pairs > 0:
                    with tc.For_i(0, pairs, 1, hint_engines=hints):
                        for _u in range(LOOP_UNROLL):
                            body()
                for _u in range(rem):
                    body()
            else:
                for _u in range(unroll):
                    body()

    nc.compile()
    _PROGRAMS[key] = nc
    return nc


def _emit_body(nc, mybir, pools, dram, consts):
    f32 = mybir.dt.float32
    f8 = mybir.dt.float8e4
    modes = _modes()
    mmdt = mybir.dt.bfloat16 if PROJ_DTYPE == "bf16" else mybir.dt.float16
    need8 = any(m == "dr" for m in modes.values())
    need16 = any(m == "16" for m in modes.values())
    ppool = pools["ppool"]
    opool = pools["opool"]
    mpool = pools["mpool"]
    proj_ps = pools["proj_ps"]
    sc_ps = pools["sc_ps"]
    vt_ps = pools["vt_ps"]
    out_ps = pools["out_ps"]
    out_d = dram["out"]
    ident = consts["ident"]
    cmaskT = consts["cmaskT"]
    w_sb = consts["w"]
    b_sb = consts["b"]

    # ---- xT loads: one tile per (dtype, s-tile); emission (= queue) order
    # is consumption order. In cast mode only the fp16 copy is DMA'd and
    # the fp8 q/k operand is produced by DVE piece-casts on device.
    cast8 = need8 and X8_SRC == "cast"
    xt8_st, xt16_st = [], []
    TW = NE * ST  # tile width in the packed layout
    for st in range(NST):
        x16eng = {"sync": nc.sync, "scalar": nc.scalar}.get(
            X16_RING, nc.scalar if st % 2 else nc.sync  # "alt"
        )
        if need8 and not cast8:
            xt = pools["xpool8"].tile([P, TW], f8, name=f"xT8_{st}", tag="xt")
            np_ = FINE if st == 0 else 1
            PW = TW // np_
            for pc in range(np_):
                nc.sync.dma_start(
                    xt[:, pc * PW : (pc + 1) * PW],
                    dram["xT8"][:, st * TW + pc * PW : st * TW + (pc + 1) * PW],
                )
            xt8_st.append(xt)
        if need16 or cast8:
            xt = pools["xpool16"].tile([P, TW], mmdt, name=f"xT16_{st}", tag="xt")
            np_ = FINE if st == 0 else (FINE16 if cast8 else 1)
            PW = TW // np_
            for pc in range(np_):
                x16eng.dma_start(
                    xt[:, pc * PW : (pc + 1) * PW],
                    dram["xT16"][:, st * TW + pc * PW : st * TW + (pc + 1) * PW],
                )
            xt16_st.append(xt)
        if cast8:
            xt8_st.append(
                pools["xpool8"].tile([P, TW], f8, name=f"x8c_{st}", tag="xt")
            )

    def emit_cast(st, sel=None):
        """DVE fp16->fp8 piece casts for tile st. sel=0: first piece only,
        sel=1: remaining pieces, None: all. Placement matters: the DVE is
        FIFO, so piece 0 goes before the v-evac (unblocks next qk early)
        and the rest after the vtrans copies."""
        if not cast8 or st >= NST:
            return
        pieces = FINE if st == 0 else FINE16
        PW = TW // pieces
        rng = {None: range(pieces), 0: range(1), 1: range(1, pieces)}[sel]
        for pc in rng:
            nc.vector.tensor_copy(
                xt8_st[st][:, pc * PW : (pc + 1) * PW],
                xt16_st[st][:, pc * PW : (pc + 1) * PW],
            )

    emit_cast(0)

    qT_sb, kT_sb, vT_sb, v_sb = consts["qkv"]
    dest = {"q": qT_sb, "k": kT_sb, "v": vT_sb}

    def emit_proj(pj, st):
        xt = xt8_st[st] if modes[pj] == "dr" else xt16_st[st]
        ps = proj_ps.tile([P, ST], f32, tag="proj")
        if modes[pj] == "dr":
            for g in range(NE // 2):
                nc.tensor.matmul(
                    ps[:],
                    lhsT=w_sb[pj][:, 2 * g * D : (2 * g + 2) * D].rearrange(
                        "p (i d) -> p i d", i=2
                    ),
                    rhs=xt[:, 2 * g * ST : (2 * g + 2) * ST].rearrange(
                        "p (i s) -> p i s", i=2
                    ),
                    start=(g == 0),
                    stop=(g == NE // 2 - 1),
                    perf_mode=mybir.MatmulPerfMode.DoubleRow,
                )
            nc.vector.tensor_scalar(
                dest[pj][:, st * ST : (st + 1) * ST],
                ps[:],
                1.0 / W_SCALE,
                b_sb[pj],
                op0=mybir.AluOpType.mult,
                op1=mybir.AluOpType.add,
            )
        else:
            for ec in range(NE):
                nc.tensor.matmul(
                    ps[:],
                    lhsT=w_sb[pj][:, ec * D : (ec + 1) * D],
                    rhs=xt[:, ec * ST : (ec + 1) * ST],
                    start=(ec == 0),
                    stop=(ec == NE - 1),
                )
            nc.vector.tensor_scalar_add(
                dest[pj][:, st * ST : (st + 1) * ST],
                ps[:],
                b_sb[pj],
            )

    probs_pieces = {}

    def emit_piece(c, j):
        qs = max(c * ST, j * P)
        w = (c + 1) * ST - qs
        sps = sc_ps.tile([P, ST], f32, tag="sc")
        diag = j * P >= c * ST
        nc.tensor.matmul(
            sps[:, :w],
            lhsT=kT_sb[:, j * P : (j + 1) * P],
            rhs=qT_sb[:, qs : qs + w],
            start=True,
            stop=not (diag and CMASK_PE),
        )
        if diag:
            if CMASK_PE:
                nc.tensor.matmul(
                    sps[:, 0:P],
                    lhsT=ident[:],
                    rhs=cmaskT[:],
                    start=False,
                    stop=True,
                )
            else:
                nc.vector.tensor_add(sps[:, 0:P], sps[:, 0:P], cmaskT[:])
        prb = ppool.tile([P, ST], mmdt, name="prb", tag="probs")
        nc.scalar.activation(
            prb[:, :w],
            sps[:, :w],
            func=mybir.ActivationFunctionType.Exp,
            bias=0.0,
            scale=SCALE,
        )
        probs_pieces[(j, c)] = (prb, qs)

    body_osb = {}

    def av_group(c, i, osbw=None):
        ops = out_ps.tile([P, D + 1], f32, tag="out")
        for j in range(i + 1):
            prb, qs = probs_pieces[(j, c)]
            off = i * P - qs
            nc.tensor.matmul(
                ops[:],
                lhsT=prb[:, off : off + P],
                rhs=v_sb[:, j * VSTRIDE : j * VSTRIDE + VW],
                start=(j == 0),
                stop=(j == i),
            )
        recip = mpool.tile([P, 1], f32, tag="recip")
        nc.vector.reciprocal(recip[:], ops[:, D : D + 1])
        eng = {"scalar": nc.scalar, "sync": nc.sync,
               "gpsimd": nc.gpsimd}[OUT_DMA]
        if osbw is not None:
            # batched stores: fewer dma_start instructions on the ring
            # (~625ns of DGE generation each) between this body's inputs
            # and the next body's
            if OSTORE == "chunk":
                g, last, nb = i - 4 * c, 3 + 4 * c, 4
                ov = out_d.rearrange("(cc i p) d -> cc p i d", i=4, p=P)[c]
            else:  # "body"
                g, last, nb = i, NS - 1, NS
                ov = out_d.rearrange("(i p) d -> p i d", p=P)
            nc.vector.tensor_scalar_mul(
                osbw[:, g * D : (g + 1) * D], ops[:, 0:D], recip[:, 0:1]
            )
            if i == last and "store" in PHASES:
                eng.dma_start(ov, osbw[:].rearrange("p (i d) -> p i d", i=nb))
            return
        osb = opool.tile([P, D], f32, tag="osb")
        nc.vector.tensor_scalar_mul(osb[:], ops[:, 0:D], recip[:, 0:1])
        if "store" in PHASES:
            eng.dma_start(out_d[i * P : (i + 1) * P, :], osb[:])

    def emit_av(c):
        if "av" not in PHASES:
            return
        if OSTORE == "chunk":
            osbw = opool.tile([P, 4 * D], f32, name="osb4", tag="osb4")
        elif OSTORE == "body":
            if c == 0:
                body_osb["t"] = opool.tile(
                    [P, NS * D], f32, name="osb16", tag="osb16"
                )
            osbw = body_osb["t"]
        else:
            osbw = None
        for i in range(4 * c, 4 * c + 4):
            av_group(c, i, osbw)

    NB = ST // P  # v blocks per s-tile

    def emit_vtrans(st):
        if "vtrans" not in PHASES:
            return
        if VT_MODE == "dma":
            # XBAR hardware transpose on the DMA engines (SBUF->SBUF,
            # 2-byte dtype): frees the PE of 4 transpose-mode matmuls and
            # the vt PSUM bank entirely
            for b_ in range(NB):
                sb = st * NB + b_
                nc.sync.dma_start_transpose(
                    v_sb[:, sb * VSTRIDE : sb * VSTRIDE + D],
                    vT_sb[:, sb * P : (sb + 1) * P],
                )
            return
        # 4 PE transposes into one PSUM bank, one strided DVE evac
        tp = vt_ps.tile([P, NB * P], mmdt, tag="vt")
        for b_ in range(NB):
            sb = st * NB + b_
            nc.tensor.transpose(
                tp[:, b_ * P : (b_ + 1) * P], vT_sb[:, sb * P : (sb + 1) * P],
                ident[:],
            )
        dstv = v_sb[:, st * NB * VSTRIDE : (st * NB + NB) * VSTRIDE].rearrange(
            "p (b w) -> p b w", b=NB
        )
        nc.vector.tensor_copy(
            dstv[:, :, 0:D], tp[:].rearrange("p (b d) -> p b d", b=NB)
        )

    def vproj_units(st, group=2):
        """v-projection as interleavable thunks (accumulation into one
        proj bank may legally interleave with matmuls to other banks);
        returns (units, evac)."""
        ps = proj_ps.tile([P, ST], f32, tag="proj")
        xt = xt16_st[st]
        units = []
        for u0 in range(0, NE, group):
            def unit(u0=u0):
                for ec in range(u0, min(u0 + group, NE)):
                    nc.tensor.matmul(
                        ps[:],
                        lhsT=w_sb["v"][:, ec * D : (ec + 1) * D],
                        rhs=xt[:, ec * ST : (ec + 1) * ST],
                        start=(ec == 0),
                        stop=(ec == NE - 1),
                    )
            units.append(unit)

        def evac():
            nc.vector.tensor_scalar_add(
                dest["v"][:, st * ST : (st + 1) * ST], ps[:], b_sb["v"]
            )

        return units, evac

    if ORDER == "avmix":
        # like avmid, but the scores pieces are interleaved at matmul
        # granularity with AV(st-1) groups and v-proj chunk units: the PE
        # queue is strict FIFO, so a block of back-to-back score matmuls
        # stalls ~350ns/piece on exp freeing score banks -- interleaved
        # filler work absorbs that
        for st in range(NST):
            c = st
            for pj in ("q", "k") if "proj" in PHASES else ():
                emit_proj(pj, st)
            emit_cast(st + 1, sel=0)
            fillers = []
            if st >= 1 and "av" in PHASES:
                fillers += [
                    (lambda i=i: av_group(st - 1, i))
                    for i in range(4 * (st - 1), 4 * (st - 1) + 4)
                ]
            vevac = None
            if "proj" in PHASES and modes["v"] == "16":
                units, vevac = vproj_units(st)
                fillers += units
            elif "proj" in PHASES:
                fillers.append(lambda: emit_proj("v", st))
            pieces = (
                [(lambda j=j: emit_piece(c, j)) for j in range(4 * c + 4)]
                if "scores" in PHASES
                else []
            )
            # two lead fillers cover the q/k PSUM-evac bubble
            lead = min(2, len(fillers))
            for f in fillers[:lead]:
                f()
            fillers = fillers[lead:]
            np_, nf = len(pieces), len(fillers)
            fi = 0
            for pi, pc_ in enumerate(pieces):
                pc_()
                want = ((pi + 1) * nf) // np_ if np_ else nf
                while fi < want:
                    fillers[fi]()
                    fi += 1
            while fi < nf:
                fillers[fi]()
                fi += 1
            if vevac is not None:
                vevac()
            emit_vtrans(st)
            emit_cast(st + 1, sel=1)
        emit_av(NST - 1)
    elif ORDER == "avmid":
        # pipelined order: AV(c) runs one s-tile late, under the next
        # tile's projections; exp(c) overlaps v/vtrans/qk instead of
        # stalling the PE before its own AV
        for st in range(NST):
            c = st
            for pj in ("q", "k") if "proj" in PHASES else ():
                emit_proj(pj, st)
            if st >= 1:
                emit_av(st - 1)
            emit_cast(st + 1, sel=0)
            for j in range(4 * c + 4) if "scores" in PHASES else ():
                emit_piece(c, j)
            for pj in ("v",) if "proj" in PHASES else ():
                emit_proj(pj, st)
            emit_vtrans(st)
            emit_cast(st + 1, sel=1)
        emit_av(NST - 1)
    else:
        for st in range(NST):
            # ---- projections for this s-tile ----
            first_projs = ("q",) if SPLIT_SCORES else ("q", "k")
            for pj in first_projs if "proj" in PHASES else ():
                emit_proj(pj, st)

            # ---- scoresT + exp for q-chunk c = st ----
            c = st
            if SPLIT_SCORES:
                for j in range(4 * c) if "scores" in PHASES else ():
                    emit_piece(c, j)
                for pj in ("k",) if "proj" in PHASES else ():
                    emit_proj(pj, st)
                for j in range(4 * c, 4 * c + 4) if "scores" in PHASES else ():
                    emit_piece(c, j)
            else:
                for j in range(4 * c + 4) if "scores" in PHASES else ():
                    emit_piece(c, j)

            for pj in ("v",) if "proj" in PHASES else ():
                emit_proj(pj, st)
            emit_vtrans(st)
            emit_av(c)
            emit_cast(st + 1)


def make_in_maps(x, Wq, bq, Wk, bk, Wv, bv):
    import ml_dtypes

    modes = _modes()
    f8np = ml_dtypes.float8_e4m3
    mm_np = ml_dtypes.bfloat16 if PROJ_DTYPE == "bf16" else np.float16
    x = np.asarray(x, dtype=np.float32)
    W = {"q": Wq, "k": Wk, "v": Wv}

    def wcast(Wm, dt_, scale):
        wt = np.asarray(Wm, dtype=np.float32).T * scale  # [E, D]
        packed = wt.reshape(NE, P, D).transpose(1, 0, 2).reshape(P, NE * D)
        return np.ascontiguousarray(packed).astype(dt_)

    shared = {
        "bias": np.ascontiguousarray(
            np.stack(
                [np.asarray(b, dtype=np.float32) for b in (bq, bk, bv)], axis=1
            )
        ),
    }
    p8 = [wcast(W[pj], f8np, W_SCALE) for pj in ("q", "k", "v") if modes[pj] == "dr"]
    p16 = [wcast(W[pj], mm_np, 1.0) for pj in ("q", "k", "v") if modes[pj] == "16"]
    if p8:
        shared["w8"] = np.ascontiguousarray(np.concatenate(p8, axis=1))
    if p16:
        shared["w16"] = np.ascontiguousarray(np.concatenate(p16, axis=1))

    maps = []
    for b in range(B):
        m = dict(shared)
        # pack xT [E, S] -> [P, NST*NE*ST]: tile st's per-partition data
        # contiguous, ec-major within a tile (matches kernel slicing)
        xt = x[b].T.reshape(NE, P, NST, ST).transpose(1, 2, 0, 3).reshape(
            P, NST * NE * ST
        )
        xt = np.ascontiguousarray(xt)
        if p8 and X8_SRC == "host":
            m["xT8"] = xt.astype(f8np)
        if p16 or (p8 and X8_SRC == "cast"):
            m["xT16"] = xt.astype(mm_np)
        maps.append(m)
    return maps


def kernel(x, Wq, bq, Wk, bk, Wv, bv):
    from concourse.bass_utils import run_bass_kernel_spmd

    nc = build_program()
    in_maps = make_in_maps(x, Wq, bq, Wk, bk, Wv, bv)
    res = run_bass_kernel_spmd(nc, in_maps, list(range(B)))
    return np.stack([res.results[i]["out"] for i in range(B)], axis=0)


# revision 49
# speedup vs baseline: 1.5706x; 1.5706x over previous
"""Single-head causal attention (B=8, S=2048, E=2048, D=128) on 8 trn2 cores.

Sharding: data-parallel over batch — one batch element per NeuronCore.

Host marshaling per core: xT = x[b].T in two precisions — fp8(e4m3) for
the q/k projections and fp16 for the v projection; q/k weights are
pre-scaled by W_SCALE, transposed and packed into one fp8 tensor in the
exact SBUF tile layout, the v weights into one fp16 tensor, the three
biases into one [128, 3] tensor.

Per-core dataflow (f32 PSUM accumulation everywhere):
  - q/k projections run fp8 DoubleRow (2 contraction chunks per matmul,
    ~1.4x PE rate); the 1/W_SCALE unscale + bias fold into the VectorE
    PSUM->SBUF evacuation. v projection runs fp16 (10-bit mantissa) —
    v quantization error hits the output directly, while q/k error is
    crushed by the 1/sqrt(2048) softmax logit scale (rel err ~3.5e-3).
  - vT is re-transposed on the PE into natural [S, D] blocks (4 per
    s-tile into one PSUM bank, one strided DVE evac), augmented with a
    ones column: the AV matmul then yields the softmax denominator for
    free as output column 128
  - scoresT[k, q] per k-block j: single fp16 matmul (K=D=128), exact
    causal trim of the q range; the diagonal 128-block's causal mask is
    accumulated on the PE (extra N=128 matmul, fp16-safe -60000);
    ScalarE computes exp(scale*s) straight out of PSUM into fp16 probsT
  - AV per q-block i accumulates probsT_j.T @ v_aug_j over j<=i in PSUM;
    VectorE takes 1/denominator and applies it while evacuating into a
    body-wide [128, 16*128] staging tile; ONE strided dma_start per body
    ships the whole natural-layout f32 output (each dma_start costs
    ~625ns of DGE ring generation that would otherwise sit between this
    body's input stream and the next body's)
  - emission order pipelines s-tiles: [qk-proj][AV of prev chunk]
    [scores][v-proj][vtrans]; AV(c) runs one s-tile late so exp(c)
    overlaps PE work instead of stalling before its own AV. For
    repeated execution, LOOP_UNROLL bodies share one For_i iteration so
    the loop's all-engine barrier fires 1/8th as often and body n+1's
    DMA/projections overlap body n's exp/AV tail.
"""

import math

import numpy as np

B = 8
S = 2048
E = 2048
D = 128
P = 128
NE = E // P  # 16 contraction chunks
NS = S // P  # 16 sequence blocks
ST = 512  # s-tile width for projections / score chunks
NST = S // ST  # 4
VW = D + 1  # logical v block width incl. ones column
VSTRIDE = D + 1  # physical stride of v blocks in SBUF
SCALE = 1.0 / math.sqrt(S)
NEG = -1.0e30

_PROGRAMS = {}

# which phases to emit (for microbenchmarking): subset of
# {"proj", "vtrans", "scores", "av", "store"}
PHASES = frozenset({"proj", "vtrans", "scores", "av", "store"})

# matmul operand precision: "fp16" (10-bit mantissa, same PE speed) /
# "bf16" / "fp8" (DoubleRow projections, 2x PE, rel err 2.95e-2: FAILS) /
# "mixed" (q/k fp8 DoubleRow + v fp16: q/k quantization error is crushed
# by the 1/sqrt(2048) softmax logit scale, v keeps fp16 accuracy ->
# rel err ~3.6e-3, ~2/3 of the all-fp8 PE win)
PROJ_DTYPE = "mixed"
W_SCALE = 256.0  # host pre-scale of W before fp8 quantization

# tunables: PSUM bank split (proj, sc, out; vt fixed at 1) and probs pool depth
PS_CFG = (2, 3, 2)
PPOOL = 32  # avmid keeps two chunks' probs live (+4 cross-body slack)
OUT_DMA = "sync"  # "sync" | "scalar": DGE queue for output stores
# (scalar steals ~667ns of Act sequencer per store from the exp stream;
#  DVE/PE cannot initiate DMAs)
SPLIT_SCORES = False  # legacy order only
XBUFS8 = 5  # fp8 x-tile pool depth (1 slot of cross-body overlap)
XBUFS16 = 5  # fp16 x-tile pool depth
FINE = 2  # first-tile DMA piece count (each dma_start costs ~625ns ring gen)
# "avmid": per st emit [qk][AV(st-1)][scores][v][vtrans], tail AV(3) --
# AV(st-1) covers the q/k PSUM-evac bubble and exp(c) overlaps v/vtrans/
# next qk instead of serializing before its own AV. "legacy": baseline.
ORDER = "avmid"
X16_RING = "alt"  # "sync" | "scalar" | "alt": DGE ring(s) for the xt16 stream
# fold the causal mask into the PE score accumulation (one extra N=128
# matmul on the diagonal piece) instead of a DVE add -- keeps the DVE off
# the scores->exp critical path. Mask value -60000 is fp16-safe and
# underflows exp to exactly 0 after the 1/sqrt(S) scale.
CMASK_PE = True
# "host": DMA a separate fp8 copy of x (12.6MB/iter total input DMA).
# "cast": DMA only the fp16 copy and produce the fp8 q/k operand with
# DVE tensor_copy casts on device (8.6MB/iter; DVE has the slack).
X8_SRC = "host"
FINE16 = 2  # DMA/cast piece count for xt16 tiles st>=1 (cast mode)
# bodies per For_i iteration: the loop's all-engine barrier fires every
# LOOP_UNROLL bodies, so body n+1's DMA stream and projections overlap
# body n's exp/AV tail. iters is decomposed as
# For_i(iters//LOOP_UNROLL) * LOOP_UNROLL + (iters % LOOP_UNROLL) bodies.
LOOP_UNROLL = 8
# "pe": transpose vT->v on the PE (transpose-mode matmul + DVE evac).
# "dma": XBAR hardware transpose on the DMA engines -- frees PE time and
# the vt PSUM bank (enables sc_ps=4 via PS_CFG=(2,4,2)).
VT_MODE = "pe"
# "block": one dma_start per 128-row output block (16/body).
# "chunk": one strided dma_start per 512-row chunk (4/body).
# "body": ONE strided dma_start for the whole body's output -- each
# dma_start costs ~625ns of DGE ring generation that sits between this
# body's input stream and the next body's.
OSTORE = "body"


def _modes():
    """per-projection matmul mode: 'dr' (fp8 DoubleRow) or '16'."""
    if PROJ_DTYPE == "fp8":
        return {"q": "dr", "k": "dr", "v": "dr"}
    if PROJ_DTYPE == "mixed":
        return {"q": "dr", "k": "dr", "v": "16"}
    return {"q": "16", "k": "16", "v": "16"}


def build_program(iters=1, unroll=1):
    global _PROGRAMS
    key = (iters, unroll, PHASES, PROJ_DTYPE, PS_CFG, PPOOL, OUT_DMA,
           SPLIT_SCORES, VSTRIDE, XBUFS8, XBUFS16, FINE, ORDER, X16_RING,
           CMASK_PE, X8_SRC, FINE16, LOOP_UNROLL, VT_MODE, OSTORE)
    if key in _PROGRAMS:
        return _PROGRAMS[key]

    import contextlib

    import concourse.bacc as bacc
    import concourse.mybir as mybir
    import concourse.tile as tile
    from concourse.masks import make_identity

    f32 = mybir.dt.float32
    f8 = mybir.dt.float8e4
    modes = _modes()
    mmdt = mybir.dt.bfloat16 if PROJ_DTYPE == "bf16" else mybir.dt.float16
    need8 = any(m == "dr" for m in modes.values())
    need16 = any(m == "16" for m in modes.values())
    n8 = sum(1 for m in modes.values() if m == "dr")
    n16 = 3 - n8

    nc = bacc.Bacc("TRN2", target_bir_lowering=False, debug=False)

    # x arrives host-packed as [P, NST*NE*ST] so each s-tile load is one
    # fully-contiguous per-partition slice (max DMA line rate)
    dram = {"out": nc.dram_tensor("out", [S, D], f32, kind="ExternalOutput")}
    if need8:
        if X8_SRC == "host":
            dram["xT8"] = nc.dram_tensor("xT8", [P, NST * NE * ST], f8, kind="ExternalInput")
        dram["w8"] = nc.dram_tensor("w8", [P, n8 * NE * D], f8, kind="ExternalInput")
    if need16 or (need8 and X8_SRC == "cast"):
        dram["xT16"] = nc.dram_tensor("xT16", [P, NST * NE * ST], mmdt, kind="ExternalInput")
    if need16:
        dram["w16"] = nc.dram_tensor("w16", [P, n16 * NE * D], mmdt, kind="ExternalInput")
    bias_d = nc.dram_tensor("bias", [D, 3], f32, kind="ExternalInput")

    with tile.TileContext(nc) as tc:
        with (
            tc.tile_pool(name="const", bufs=1) as cpool,
            tc.tile_pool(name="xt8", bufs=XBUFS8) as xpool8,
            tc.tile_pool(name="xt16", bufs=XBUFS16) as xpool16,
            tc.tile_pool(name="qkv", bufs=1) as qkvpool,
            tc.tile_pool(name="probs", bufs=PPOOL) as ppool,
            tc.tile_pool(name="osb", bufs=2) as opool,
            tc.tile_pool(name="misc", bufs=2) as mpool,
            tc.tile_pool(name="proj_ps", bufs=PS_CFG[0], space="PSUM") as proj_ps,
            tc.tile_pool(name="sc_ps", bufs=PS_CFG[1], space="PSUM") as sc_ps,
            tc.tile_pool(name="vt_ps", bufs=1, space="PSUM") as vt_ps,
            tc.tile_pool(name="out_ps", bufs=PS_CFG[2], space="PSUM") as out_ps,
        ):
            # ---- iteration-invariant setup ----
            ident = cpool.tile([P, P], mmdt, tag="ident")
            make_identity(nc, ident[:])
            # cmaskT[k_local, q_local]: 0 where q >= k (valid), NEG where q < k
            cmaskT = cpool.tile([P, P], mmdt if CMASK_PE else f32, tag="cmaskT")
            nc.gpsimd.memset(cmaskT[:], 0.0)
            nc.gpsimd.affine_select(
                out=cmaskT[:],
                in_=cmaskT[:],
                compare_op=mybir.AluOpType.is_ge,
                fill=-60000.0 if CMASK_PE else NEG,
                base=0,
                # iota[r, c] = c - r ; keep (0.0) where c - r >= 0
                pattern=[[1, P]],
                channel_multiplier=-1,
            )

            w_sb = {}
            b_sb = {}
            i8 = i16 = 0
            if need8:
                w8_sb = cpool.tile([P, n8 * NE * D], f8, tag="w8")
                nc.sync.dma_start(w8_sb[:], dram["w8"][:, :])
            if need16:
                w16_sb = cpool.tile([P, n16 * NE * D], mmdt, tag="w16")
                nc.sync.dma_start(w16_sb[:], dram["w16"][:, :])
            for pj in ("q", "k", "v"):
                if modes[pj] == "dr":
                    w_sb[pj] = w8_sb[:, i8 * NE * D : (i8 + 1) * NE * D]
                    i8 += 1
                else:
                    w_sb[pj] = w16_sb[:, i16 * NE * D : (i16 + 1) * NE * D]
                    i16 += 1
            bias_sb = cpool.tile([P, 3], f32, tag="bias")
            nc.sync.dma_start(bias_sb[:], bias_d[:, :])
            for n, pj in enumerate(("q", "k", "v")):
                b_sb[pj] = bias_sb[:, n : n + 1]

            # qkv destination tiles persist across iterations (bufs=1);
            # the ones column of v_aug is iteration-invariant -> init once
            qT_sb = qkvpool.tile([P, S], mmdt, tag="qT")
            kT_sb = qkvpool.tile([P, S], mmdt, tag="kT")
            vT_sb = qkvpool.tile([P, S], mmdt, tag="vT")
            v_sb = qkvpool.tile([P, NS * VSTRIDE], mmdt, tag="v")
            for sb in range(NS):
                nc.vector.memset(
                    v_sb[:, sb * VSTRIDE + D : sb * VSTRIDE + D + 1], 1.0
                )

            # HAM warmup while the first DMAs land (runs once, cold)
            wps = proj_ps.tile([P, ST], f32, name="warm", tag="proj")
            for wi in range(40):
                nc.tensor.matmul(
                    wps[:, 0:P],
                    lhsT=ident[:],
                    rhs=ident[:],
                    start=(wi == 0),
                    stop=(wi == 39),
                )

            def body():
                _emit_body(
                    nc,
                    mybir,
                    pools={
                        "xpool8": xpool8,
                        "xpool16": xpool16,
                        "qkvpool": qkvpool,
                        "ppool": ppool,
                        "opool": opool,
                        "mpool": mpool,
                        "proj_ps": proj_ps,
                        "sc_ps": sc_ps,
                        "vt_ps": vt_ps,
                        "out_ps": out_ps,
                    },
                    dram=dram,
                    consts={
                        "ident": ident,
                        "cmaskT": cmaskT,
                        "w": w_sb,
                        "b": b_sb,
                        "qkv": (qT_sb, kT_sb, vT_sb, v_sb),
                    },
                )

            hints = (
                mybir.EngineType.PE,
                mybir.EngineType.Activation,
                mybir.EngineType.DVE,
                mybir.EngineType.SP,
                mybir.EngineType.Pool,
            )
            if iters > 1:
                # LOOP_UNROLL bodies per hardware-loop iteration: For_i's
                # all-engine barrier fires once per group, so body n+1's
                # DMA/projections overlap body n's exp/AV tail
                pairs, rem = divmod(iters, LOOP_UNROLL)
                if pairs > 0:
                    with tc.For_i(0, pairs, 1, hint_engines=hints):
                        for _u in range(LOOP_UNROLL):
                            body()
                for _u in range(rem):
                    body()
            else:
                for _u in range(unroll):
                    body()

    nc.compile()
    _PROGRAMS[key] = nc
    return nc


def _emit_body(nc, mybir, pools, dram, consts):
    f32 = mybir.dt.float32
    f8 = mybir.dt.float8e4
    modes = _modes()
    mmdt = mybir.dt.bfloat16 if PROJ_DTYPE == "bf16" else mybir.dt.float16
    need8 = any(m == "dr" for m in modes.values())
    need16 = any(m == "16" for m in modes.values())
    ppool = pools["ppool"]
    opool = pools["opool"]
    mpool = pools["mpool"]
    proj_ps = pools["proj_ps"]
    sc_ps = pools["sc_ps"]
    vt_ps = pools["vt_ps"]
    out_ps = pools["out_ps"]
    out_d = dram["out"]
    ident = consts["ident"]
    cmaskT = consts["cmaskT"]
    w_sb = consts["w"]
    b_sb = consts["b"]

    # ---- xT loads: one tile per (dtype, s-tile); emission (= queue) order
    # is consumption order. In cast mode only the fp16 copy is DMA'd and
    # the fp8 q/k operand is produced by DVE piece-casts on device.
    cast8 = need8 and X8_SRC == "cast"
    xt8_st, xt16_st = [], []
    TW = NE * ST  # tile width in the packed layout
    for st in range(NST):
        x16eng = {"sync": nc.sync, "scalar": nc.scalar}.get(
            X16_RING, nc.scalar if st % 2 else nc.sync  # "alt"
        )
        if need8 and not cast8:
            xt = pools["xpool8"].tile([P, TW], f8, name=f"xT8_{st}", tag="xt")
            np_ = FINE if st == 0 else 1
            PW = TW // np_
            for pc in range(np_):
                nc.sync.dma_start(
                    xt[:, pc * PW : (pc + 1) * PW],
                    dram["xT8"][:, st * TW + pc * PW : st * TW + (pc + 1) * PW],
                )
            xt8_st.append(xt)
        if need16 or cast8:
            xt = pools["xpool16"].tile([P, TW], mmdt, name=f"xT16_{st}", tag="xt")
            np_ = FINE if st == 0 else (FINE16 if cast8 else 1)
            PW = TW // np_
            for pc in range(np_):
                x16eng.dma_start(
                    xt[:, pc * PW : (pc + 1) * PW],
                    dram["xT16"][:, st * TW + pc * PW : st * TW + (pc + 1) * PW],
                )
            xt16_st.append(xt)
        if cast8:
            xt8_st.append(
                pools["xpool8"].tile([P, TW], f8, name=f"x8c_{st}", tag="xt")
            )

    def emit_cast(st, sel=None):
        """DVE fp16->fp8 piece casts for tile st. sel=0: first piece only,
        sel=1: remaining pieces, None: all. Placement matters: the DVE is
        FIFO, so piece 0 goes before the v-evac (unblocks next qk early)
        and the rest after the vtrans copies."""
        if not cast8 or st >= NST:
            return
        pieces = FINE if st == 0 else FINE16
        PW = TW // pieces
        rng = {None: range(pieces), 0: range(1), 1: range(1, pieces)}[sel]
        for pc in rng:
            nc.vector.tensor_copy(
                xt8_st[st][:, pc * PW : (pc + 1) * PW],
                xt16_st[st][:, pc * PW : (pc + 1) * PW],
            )

    emit_cast(0)

    qT_sb, kT_sb, vT_sb, v_sb = consts["qkv"]
    dest = {"q": qT_sb, "k": kT_sb, "v": vT_sb}

    def emit_proj(pj, st):
        xt = xt8_st[st] if modes[pj] == "dr" else xt16_st[st]
        ps = proj_ps.tile([P, ST], f32, tag="proj")
        if modes[pj] == "dr":
            for g in range(NE // 2):
                nc.tensor.matmul(
                    ps[:],
                    lhsT=w_sb[pj][:, 2 * g * D : (2 * g + 2) * D].rearrange(
                        "p (i d) -> p i d", i=2
                    ),
                    rhs=xt[:, 2 * g * ST : (2 * g + 2) * ST].rearrange(
                        "p (i s) -> p i s", i=2
                    ),
                    start=(g == 0),
                    stop=(g == NE // 2 - 1),
                    perf_mode=mybir.MatmulPerfMode.DoubleRow,
                )
            nc.vector.tensor_scalar(
                dest[pj][:, st * ST : (st + 1) * ST],
                ps[:],
                1.0 / W_SCALE,
                b_sb[pj],
                op0=mybir.AluOpType.mult,
                op1=mybir.AluOpType.add,
            )
        else:
            for ec in range(NE):
                nc.tensor.matmul(
                    ps[:],
                    lhsT=w_sb[pj][:, ec * D : (ec + 1) * D],
                    rhs=xt[:, ec * ST : (ec + 1) * ST],
                    start=(ec == 0),
                    stop=(ec == NE - 1),
                )
            nc.vector.tensor_scalar_add(
                dest[pj][:, st * ST : (st + 1) * ST],
                ps[:],
                b_sb[pj],
            )

    probs_pieces = {}

    def emit_piece(c, j):
        qs = max(c * ST, j * P)
        w = (c + 1) * ST - qs
        sps = sc_ps.tile([P, ST], f32, tag="sc")
        diag = j * P >= c * ST
        nc.tensor.matmul(
            sps[:, :w],
            lhsT=kT_sb[:, j * P : (j + 1) * P],
            rhs=qT_sb[:, qs : qs + w],
            start=True,
            stop=not (diag and CMASK_PE),
        )
        if diag:
            if CMASK_PE:
                nc.tensor.matmul(
                    sps[:, 0:P],
                    lhsT=ident[:],
                    rhs=cmaskT[:],
                    start=False,
                    stop=True,
                )
            else:
                nc.vector.tensor_add(sps[:, 0:P], sps[:, 0:P], cmaskT[:])
        prb = ppool.tile([P, ST], mmdt, name="prb", tag="probs")
        nc.scalar.activation(
            prb[:, :w],
            sps[:, :w],
            func=mybir.ActivationFunctionType.Exp,
            bias=0.0,
            scale=SCALE,
        )
        probs_pieces[(j, c)] = (prb, qs)

    body_osb = {}

    def av_group(c, i, osbw=None):
        ops = out_ps.tile([P, D + 1], f32, tag="out")
        for j in range(i + 1):
            prb, qs = probs_pieces[(j, c)]
            off = i * P - qs
            nc.tensor.matmul(
                ops[:],
                lhsT=prb[:, off : off + P],
                rhs=v_sb[:, j * VSTRIDE : j * VSTRIDE + VW],
                start=(j == 0),
                stop=(j == i),
            )
        recip = mpool.tile([P, 1], f32, tag="recip")
        nc.vector.reciprocal(recip[:], ops[:, D : D + 1])
        eng = {"scalar": nc.scalar, "sync": nc.sync,
               "gpsimd": nc.gpsimd}[OUT_DMA]
        if osbw is not None:
            # batched stores: fewer dma_start instructions on the ring
            # (~625ns of DGE generation each) between this body's inputs
            # and the next body's
            if OSTORE == "chunk":
                g, last, nb = i - 4 * c, 3 + 4 * c, 4
                ov = out_d.rearrange("(cc i p) d -> cc p i d", i=4, p=P)[c]
            else:  # "body"
                g, last, nb = i, NS - 1, NS
                ov = out_d.rearrange("(i p) d -> p i d", p=P)
            nc.vector.tensor_scalar_mul(
                osbw[:, g * D : (g + 1) * D], ops[:, 0:D], recip[:, 0:1]
            )
            if i == last and "store" in PHASES:
                eng.dma_start(ov, osbw[:].rearrange("p (i d) -> p i d", i=nb))
            return
        osb = opool.tile([P, D], f32, tag="osb")
        nc.vector.tensor_scalar_mul(osb[:], ops[:, 0:D], recip[:, 0:1])
        if "store" in PHASES:
            eng.dma_start(out_d[i * P : (i + 1) * P, :], osb[:])

    def emit_av(c):
        if "av" not in PHASES:
            return
        if OSTORE == "chunk":
            osbw = opool.tile([P, 4 * D], f32, name="osb4", tag="osb4")
        elif OSTORE == "body":
            if c == 0:
                body_osb["t"] = opool.tile(
                    [P, NS * D], f32, name="osb16", tag="osb16"
                )
            osbw = body_osb["t"]
        else:
            osbw = None
        for i in range(4 * c, 4 * c + 4):
            av_group(c, i, osbw)

    NB = ST // P  # v blocks per s-tile

    def emit_vtrans(st):
        if "vtrans" not in PHASES:
            return
        if VT_MODE == "dma":
            # XBAR hardware transpose on the DMA engines (SBUF->SBUF,
            # 2-byte dtype): frees the PE of 4 transpose-mode matmuls and
            # the vt PSUM bank entirely
            for b_ in range(NB):
                sb = st * NB + b_
                nc.sync.dma_start_transpose(
                    v_sb[:, sb * VSTRIDE : sb * VSTRIDE + D],
                    vT_sb[:, sb * P : (sb + 1) * P],
                )
            return
        # 4 PE transposes into one PSUM bank, one strided DVE evac
        tp = vt_ps.tile([P, NB * P], mmdt, tag="vt")
        for b_ in range(NB):
            sb = st * NB + b_
            nc.tensor.transpose(
                tp[:, b_ * P : (b_ + 1) * P], vT_sb[:, sb * P : (sb + 1) * P],
                ident[:],
            )
        dstv = v_sb[:, st * NB * VSTRIDE : (st * NB + NB) * VSTRIDE].rearrange(
            "p (b w) -> p b w", b=NB
        )
        nc.vector.tensor_copy(
            dstv[:, :, 0:D], tp[:].rearrange("p (b d) -> p b d", b=NB)
        )

    def vproj_units(st, group=2):
        """v-projection as interleavable thunks (accumulation into one
        proj bank may legally interleave with matmuls to other banks);
        returns (units, evac)."""
        ps = proj_ps.tile([P, ST], f32, tag="proj")
        xt = xt16_st[st]
        units = []
        for u0 in range(0, NE, group):
            def unit(u0=u0):
                for ec in range(u0, min(u0 + group, NE)):
                    nc.tensor.matmul(
                        ps[:],
                        lhsT=w_sb["v"][:, ec * D : (ec + 1) * D],
                        rhs=xt[:, ec * ST : (ec + 1) * ST],
                        start=(ec == 0),
                        stop=(ec == NE - 1),
                    )
            units.append(unit)

        def evac():
            nc.vector.tensor_scalar_add(
                dest["v"][:, st * ST : (st + 1) * ST], ps[:], b_sb["v"]
            )

        return units, evac

    if ORDER == "avmix":
        # like avmid, but the scores pieces are interleaved at matmul
        # granularity with AV(st-1) groups and v-proj chunk units: the PE
        # queue is strict FIFO, so a block of back-to-back score matmuls
        # stalls ~350ns/piece on exp freeing score banks -- interleaved
        # filler work absorbs that
        for st in range(NST):
            c = st
            for pj in ("q", "k") if "proj" in PHASES else ():
                emit_proj(pj, st)
            emit_cast(st + 1, sel=0)
            fillers = []
            if st >= 1 and "av" in PHASES:
                fillers += [
                    (lambda i=i: av_group(st - 1, i))
                    for i in range(4 * (st - 1), 4 * (st - 1) + 4)
                ]
            vevac = None
            if "proj" in PHASES and modes["v"] == "16":
                units, vevac = vproj_units(st)
                fillers += units
            elif "proj" in PHASES:
                fillers.append(lambda: emit_proj("v", st))
            pieces = (
                [(lambda j=j: emit_piece(c, j)) for j in range(4 * c + 4)]
                if "scores" in PHASES
                else []
            )
            # two lead fillers cover the q/k PSUM-evac bubble
            lead = min(2, len(fillers))
            for f in fillers[:lead]:
                f()
            fillers = fillers[lead:]
            np_, nf = len(pieces), len(fillers)
            fi = 0
            for pi, pc_ in enumerate(pieces):
                pc_()
                want = ((pi + 1) * nf) // np_ if np_ else nf
                while fi < want:
                    fillers[fi]()
                    fi += 1
            while fi < nf:
                fillers[fi]()
                fi += 1
            if vevac is not None:
                vevac()
            emit_vtrans(st)
            emit_cast(st + 1, sel=1)
        emit_av(NST - 1)
    elif ORDER == "avmid":
        # pipelined order: AV(c) runs one s-tile late, under the next
        # tile's projections; exp(c) overlaps v/vtrans/qk instead of
        # stalling the PE before its own AV
        for st in range(NST):
            c = st
            for pj in ("q", "k") if "proj" in PHASES else ():
                emit_proj(pj, st)
            if st >= 1:
                emit_av(st - 1)
            emit_cast(st + 1, sel=0)
            for j in range(4 * c + 4) if "scores" in PHASES else ():
                emit_piece(c, j)
            for pj in ("v",) if "proj" in PHASES else ():
                emit_proj(pj, st)
            emit_vtrans(st)
            emit_cast(st + 1, sel=1)
        emit_av(NST - 1)
    else:
        for st in range(NST):
            # ---- projections for this s-tile ----
            first_projs = ("q",) if SPLIT_SCORES else ("q", "k")
            for pj in first_projs if "proj" in PHASES else ():
                emit_proj(pj, st)

            # ---- scoresT + exp for q-chunk c = st ----
            c = st
            if SPLIT_SCORES:
                for j in range(4 * c) if "scores" in PHASES else ():
                    emit_piece(c, j)
                for pj in ("k",) if "proj" in PHASES else ():
                    emit_proj(pj, st)
                for j in range(4 * c, 4 * c + 4) if "scores" in PHASES else ():
                    emit_piece(c, j)
            else:
                for j in range(4 * c + 4) if "scores" in PHASES else ():
                    emit_piece(c, j)

            for pj in ("v",) if "proj" in PHASES else ():
                emit_proj(pj, st)
            emit_vtrans(st)
            emit_av(c)
            emit_cast(st + 1)


def make_in_maps(x, Wq, bq, Wk, bk, Wv, bv):
    import ml_dtypes

    modes = _modes()
    f8np = ml_dtypes.float8_e4m3
    mm_np = ml_dtypes.bfloat16 if PROJ_DTYPE == "bf16" else np.float16
    x = np.asarray(x, dtype=np.float32)
    W = {"q": Wq, "k": Wk, "v": Wv}

    def wcast(Wm, dt_, scale):
        wt = np.asarray(Wm, dtype=np.float32).T * scale  # [E, D]
        packed = wt.reshape(NE, P, D).transpose(1, 0, 2).reshape(P, NE * D)
        return np.ascontiguousarray(packed).astype(dt_)

    shared = {
        "bias": np.ascontiguousarray(
            np.stack(
                [np.asarray(b, dtype=np.float32) for b in (bq, bk, bv)], axis=1
            )
        ),
    }
    p8 = [wcast(W[pj], f8np, W_SCALE) for pj in ("q", "k", "v") if modes[pj] == "dr"]
    p16 = [wcast(W[pj], mm_np, 1.0) for pj in ("q", "k", "v") if modes[pj] == "16"]
    if p8:
        shared["w8"] = np.ascontiguousarray(np.concatenate(p8, axis=1))
    if p16:
        shared["w16"] = np.ascontiguousarray(np.concatenate(p16, axis=1))

    maps = []
    for b in range(B):
        m = dict(shared)
        # pack xT [E, S] -> [P, NST*NE*ST]: tile st's per-partition data
        # contiguous, ec-major within a tile (matches kernel slicing)
        xt = x[b].T.reshape(NE, P, NST, ST).transpose(1, 2, 0, 3).reshape(
            P, NST * NE * ST
        )
        xt = np.ascontiguousarray(xt)
        if p8 and X8_SRC == "host":
            m["xT8"] = xt.astype(f8np)
        if p16 or (p8 and X8_SRC == "cast"):
            m["xT16"] = xt.astype(mm_np)
        maps.append(m)
    return maps


def kernel(x, Wq, bq, Wk, bk, Wv, bv):
    from concourse.bass_utils import run_bass_kernel_spmd

    nc = build_program()
    in_maps = make_in_maps(x, Wq, bq, Wk, bk, Wv, bv)
    res = run_bass_kernel_spmd(nc, in_maps, list(range(B)))
    return np.stack([res.results[i]["out"] for i in range(B)], axis=0)
